# revision 73
# baseline (speedup 1.0000x reference)
"""Trainium2 Bass kernel for nn_Attention_block (retrieval_knn).

Reference (per sample b, match A in {Q_flo, K_dep}, V = V_rgb):
  T[i,j] = <A[:,i], V[:,j]>          [4096, 4096] score matrix
  S[j] = max_i T ; idx[j] = argmax_i T
  C = conv1x1([V; A[:, idx]]) * S    (conv1: 128->64)
  fused = [C_v, C_k, V]              (192 ch)
  y = relu(BN(conv3x3(fused)))       (conv2: 192->64, pad 1)

Sharding: 8 cores = 4 samples x 2 W-halves (pure data parallel; each core
takes a 1-row halo each side of its half for the 3x3 conv and computes its
2176 j-columns against the full 4096-long i axis).

Device-side structure per core (PSUM-resident argmax spine; no ACT
evacuation pass):
  - G-trick: gather commutes with conv1's TA half:
      conv1([V;TA]) + b1 = W1v@V + (W1t@A + b1)[:, idx]
    G' = W1t@A + b1 is computed once per match, transposed into DRAM
    [4096, 64], and argmax rows are fetched by indirect-DMA gather.
  - Scores must be fp32-exact: smallest top-2 score gap here is ~1.5e-4 and
    the reference argmax is f32.  Plain fp32 matmuls cost 4 PE cycles/row
    and fp32r is TF32 (10-bit - flips argmaxes), so the spine uses an exact
    bf16 split computed on the host: A = A1 + A2, V = V1 + V2 (hi/lo bf16
    pairs); T = V1'A1 + (V1'A2 + V2'A1), where the two cross terms fold
    into ONE K=128 matmul by stacking [V1;V2] against [A2;A1] (PE cost is
    per output row, independent of K) - 2 bf16 matmuls/chunk instead of 1
    fp32 at 4 cyc/row.  |T~ - T| ~ 3e-5, 5x below the min gap.
  - Argmax spine, m-major per 128-j tile: T never leaves PSUM (killing the
    141us ACT evacuation pass of the previous design).  Each [128,1024]
    PSUM quarter (2 chunk-matmul pairs) is consumed by a chained DVE
    prefix-max scan (tensor_tensor_scan max/bypass, initial = previous
    quarter's last column, an AP) writing an SBUF scratch row P [128,4096];
    data1 points at a pre-zeroed dummy since src0/src1 cannot both be PSUM.
    S = P[:,-1] (Pool copies it into s_all, and a per-tile partition-major
    DMA streams it to s_dram for the group's s_bc broadcast), and
    idx = #(P_i < S): one ACT Sign pass (scale=-1, bias=S) with the sum
    accumulator - exact, first-occurrence ties.  The last j-tile counts on
    DVE instead (2x-mode is_lt tensor_scalar, 2.2us vs 3.8us) so the drain
    tail isn't gated on the ACT queue.  (Pool/gpsimd cannot run any
    TensorScalarPtr-class op - scans included - on the real compile path:
    walrus rejects them, so Pool only gets gathers/copies/memsets.)
    Engine busy: DVE ~172us (bottleneck), PE ~147us, ACT ~140us, Pool ~45.
  - Fine-grained software pipelining: all non-spine PE work (G' chunks and
    transposes, conv1 per-match passes, conv2 single taps, s_group
    broadcasts) is chopped into ~0.2-0.9us closures on a deque; every spine
    quarter banks 340ns of allowance and pops fully-funded units, so PE
    stays packed inside the scans' per-quarter slack and never head-blocks
    the DVE scan cadence behind a multi-us conv excursion.  Gathers are
    deferred 2 match-tiles; the drain tail is hand-rolled with conv2
    quarter 2 interleaved into the last chunk's latency chain to hold the
    fast PE p-state.  ORDERING IS CORRECTNESS-CRITICAL in two places the
    Tile framework does not track: gt_dram (G' transposes' DMAs must land
    before the first gathers fire at the g==1 flush) and fa3 (conv1 chunk
    q+1's units must be emitted before conv2 quarter q's taps).
  - conv1 / conv2 / G' matmuls run in bf16 (value paths; ~2.9e-3 rel err,
    well under the 2e-2 gate).  The G'->transpose->gather->conv1-TA chain
    stays fp32 (transposes must dtype-match their data, and a 16-bit
    transpose output cannot accumulate into an fp32 PSUM group).
  - PSUM (8 banks): spine 3x[128,1024] quarters rotating (6) + cva (conv1 +
    G' chunks, per-match sequential) + c2 (conv2 + G' transpose staging).
    Sharing one bank between two concurrent 64-partition accumulation
    groups passes the simulator but breaks on hardware (start=True zeroes
    the whole region) - sequential per-match reuse with full evacuation
    between matches is safe and is what cva/c2 do.
"""

import collections

import numpy as np
import ml_dtypes

import concourse.bass as bass
import concourse.bacc as bacc
import concourse.mybir as mybir
from concourse.tile import TileContext
from concourse import bass_utils
from concourse.masks import make_identity

F32 = mybir.dt.float32
BF16 = mybir.dt.bfloat16
I32 = mybir.dt.int32
AF = mybir.ActivationFunctionType
OP = mybir.AluOpType

B, C, W, H = 4, 64, 64, 64
HW = W * H                     # 4096
BN_EPS = 1e-5
N_CORES = 8
WROWS = W // 2 + 2             # 34 window rows (half + 1-row halo each side)
JW = WROWS * H                 # 2176 j-columns per core
JT = JW // 128                 # 17 j-tiles
NCH = HW // 512                # 8 i-chunks
OUT_ROWS = W // 2              # 32 interior rows per core
YPAD = H + 2                   # 66 padded y positions in fused layout

NEG = -3.0e38


def _build_nc():
    nc = bacc.Bacc("TRN2", target_bir_lowering=False)

    # Exact bf16 split pairs (X = X1 + X2 with X2 itself bf16-exact to
    # ~2^-17 rel).  T = V1'A1 + (V1'A2 + V2'A1): the two cross terms are
    # computed as ONE K=128 matmul by stacking [V1;V2] against [A2;A1] -
    # PE cost is per output row, independent of K.
    #   ahi: rows 0:64 = A1_q, rows 64:128 = A1_k
    #   axq/axk: rows 0:64 = A2_m, rows 64:128 = A1_m  (match m)
    #   vhi: V1 duplicated on both row halves; vx: rows 0:64 V1, 64:128 V2
    ahi = nc.dram_tensor("ahi", [128, HW], BF16, kind="ExternalInput")
    axq = nc.dram_tensor("axq", [128, HW], BF16, kind="ExternalInput")
    axk = nc.dram_tensor("axk", [128, HW], BF16, kind="ExternalInput")
    vhi = nc.dram_tensor("vhi", [128, JW], BF16, kind="ExternalInput")
    vx = nc.dram_tensor("vx", [128, JW], BF16, kind="ExternalInput")
    vwin = nc.dram_tensor("vwin", [C, JW], BF16, kind="ExternalInput")
    w1vt = nc.dram_tensor("w1vt", [C, C], BF16, kind="ExternalInput")
    w1tt = nc.dram_tensor("w1tt", [128, C], BF16, kind="ExternalInput")
    b1d = nc.dram_tensor("b1d", [C, 1], F32, kind="ExternalInput")
    w2ad = nc.dram_tensor("w2ad", [128, 9 * C], BF16, kind="ExternalInput")
    w2bd = nc.dram_tensor("w2bd", [C, 9 * C], BF16, kind="ExternalInput")
    bnad = nc.dram_tensor("bnad", [128, 1], F32, kind="ExternalInput")
    bnbd = nc.dram_tensor("bnbd", [128, 1], F32, kind="ExternalInput")
    yout = nc.dram_tensor("y", [C, OUT_ROWS * H], F32, kind="ExternalOutput")

    if True:
      with TileContext(nc) as tc:
        with tc.tile_pool(name="persist", bufs=1) as pp:
            ahi_t = pp.tile([128, HW], BF16)
            axq_t = pp.tile([128, HW], BF16)
            axk_t = pp.tile([128, HW], BF16)
            vhi_t = pp.tile([128, JW], BF16)
            vx_t = pp.tile([128, JW], BF16)
            w1vt_t = pp.tile([C, C], BF16)
            w1tt_t = pp.tile([128, C], BF16)
            b1_t = pp.tile([C, 1], F32)
            w2a_t = pp.tile([128, 9 * C], BF16)
            w2b_t = pp.tile([C, 9 * C], BF16)
            bna_t = pp.tile([128, 1], F32)
            bnb_t = pp.tile([128, 1], F32)
            ident = pp.tile([128, 128], F32)
            gtile = [pp.tile([128, JT * C], F32, tag="gtq", name="gtq_t"),
                     pp.tile([128, JT * C], F32, tag="gtk", name="gtk_t")]
            s_all = [pp.tile([128, JT], F32, tag="sq", name="sq_t"),
                     pp.tile([128, JT], F32, tag="sk", name="sk_t")]
            idx_all = [pp.tile([128, JT], I32, tag="idxq", name="idxq_t"),
                       pp.tile([128, JT], I32, tag="idxk", name="idxk_t")]
            s_bc = pp.tile([128, JW], F32)     # rows 0:64 Sq, 64:128 Sk
            fused_a = pp.tile([128, WROWS * YPAD], BF16)  # C_v / C_k
            fused_b = pp.tile([C, WROWS * YPAD], BF16)    # V, y-padded
            out_sb = pp.tile([128, OUT_ROWS * H // 2], F32)

            # SP.SEQ issues one DMA per ~650ns, so the head of the queue is
            # precious: the first spine quarter's operands go first (vhi/vx
            # slices ride the ACT queue in parallel), then everything else
            # as few big transfers.
            nc.sync.dma_start(out=w1tt_t[:], in_=w1tt[:])
            nc.sync.dma_start(out=b1_t[:], in_=b1d[:])
            nc.sync.dma_start(out=ahi_t[:, 0:1024], in_=ahi[:, 0:1024])
            nc.sync.dma_start(out=axq_t[:, 0:1024], in_=axq[:, 0:1024])
            nc.sync.dma_start(out=vhi_t[:, 0:256], in_=vhi[:, 0:256])
            nc.sync.dma_start(out=vx_t[:, 0:256], in_=vx[:, 0:256])
            nc.sync.dma_start(out=axk_t[:, 0:1024], in_=axk[:, 0:1024])
            for cq in range(1, 4):
                cs = slice(cq * 1024, (cq + 1) * 1024)
                nc.sync.dma_start(out=ahi_t[:, cs], in_=ahi[:, cs])
                nc.sync.dma_start(out=axq_t[:, cs], in_=axq[:, cs])
                nc.sync.dma_start(out=axk_t[:, cs], in_=axk[:, cs])
            nc.sync.dma_start(out=vhi_t[:, 256:JW], in_=vhi[:, 256:JW])
            nc.sync.dma_start(out=vx_t[:, 256:JW], in_=vx[:, 256:JW])
            nc.sync.dma_start(out=w1vt_t[:], in_=w1vt[:])
            nc.sync.dma_start(out=w2a_t[:], in_=w2ad[:])
            nc.sync.dma_start(out=w2b_t[:], in_=w2bd[:])
            nc.sync.dma_start(out=bna_t[:], in_=bnad[:])
            nc.sync.dma_start(out=bnb_t[:], in_=bnbd[:])
            make_identity(nc, ident[:])

            # the first DVE scan reads scan_dummy (bypassed data1): its
            # memset must head the Pool queue, ahead of the big gtile/fused
            # memsets, or the spine start stalls ~5us behind them
            scan_dummy = pp.tile([128, 1024], F32)
            nc.gpsimd.memset(scan_dummy[:], 0.0)

            fb3 = fused_b[:].rearrange("c (x y) -> c x y", y=YPAD)
            nc.gpsimd.memset(fused_b[:], 0.0)
            nc.sync.dma_start(
                out=fb3[:, :, 1:H + 1],
                in_=vwin[:].rearrange("c (x y) -> c x y", y=H))
            nc.gpsimd.memset(gtile[0][:], 0.0)
            nc.gpsimd.memset(gtile[1][:], 0.0)

            with tc.tile_pool(name="gdram", bufs=1, space="DRAM") as gdr:
                gt_dram = [gdr.tile([HW, C], F32, tag="gtdq", name="gtdq_t"),
                           gdr.tile([HW, C], F32, tag="gtdk", name="gtdk_t")]

                # ---- Phases 3-6 interleaved ----
                fa3 = fused_a[:].rearrange("c (x y) -> c x y", y=YPAD)
                nc.gpsimd.memset(fa3[:, :, 0:1], 0.0)
                nc.gpsimd.memset(fa3[:, :, YPAD - 1:YPAD], 0.0)
                with tc.tile_pool(name="sp_ps", bufs=1, space="PSUM") as sps, \
                     tc.tile_pool(name="sp_sb", bufs=2) as ssb, \
                     tc.tile_pool(name="sp_sm", bufs=4) as ssm, \
                     tc.tile_pool(name="cv_ps", bufs=1, space="PSUM") as cvp, \
                     tc.tile_pool(name="s4_sb", bufs=2) as s4, \
                     tc.tile_pool(name="s4_dram", bufs=1, space="DRAM") as d4:

                    NSCR = 4

                    # ---- Phase 1+2: G' = W1t @ A1 + b1; transpose to DRAM.
                    # Emitted as closures interleaved into spine group 0 so
                    # the startup isn't serial (spine doesn't depend on G').
                    g_sbs = [pp.tile([C, HW], F32, tag="gsbq", name="gsbq"),
                             pp.tile([C, HW], F32, tag="gsbk", name="gsbk")]

                    def g_chunk(c8, m):
                        # sequential per-match through one PSUM bank (the
                        # two 64-partition halves cannot share a bank
                        # concurrently: start=True zeroes the whole region)
                        pm = cvp.tile([C, 512], F32, tag="cva", name="gm")
                        ro = m * C
                        nc.tensor.matmul(
                            pm[:], w1tt_t[ro:ro + C, :],
                            ahi_t[ro:ro + C, c8 * 512:(c8 + 1) * 512],
                            start=True, stop=True,
                            tile_position=(ro, 0))
                        nc.scalar.activation(
                            g_sbs[m][:, c8 * 512:(c8 + 1) * 512],
                            pm[:],
                            AF.Identity, bias=b1_t[:, 0:1], scale=1.0)

                    def g_transpose(m, grp):
                        g_sb = g_sbs[m]
                        pst = cvp.tile([128, 512], F32, tag="c2",
                                       name="gtr")
                        stg = pp.tile([128, 512], F32, tag="stg")
                        for t in range(8):
                            blk = grp * 8 + t
                            nc.tensor.matmul(
                                pst[:, t * C:(t + 1) * C],
                                g_sb[:, blk * 128:(blk + 1) * 128],
                                ident[0:C, 0:C], is_transpose=True,
                                start=True, stop=True)
                        nc.scalar.copy(stg[:], pst[:])
                        nc.sync.dma_start(
                            out=gt_dram[m][:]
                            .rearrange("(g p) c -> p g c", p=128)
                            [:, grp * 8:(grp + 1) * 8, :],
                            in_=stg[:].rearrange("p (g c) -> p g c", c=C))

                    # Fine-grained software pipelining: every non-spine PE
                    # task (G', conv1, s_group, conv2) is chopped into ~0.5us
                    # units on a deque; each spine quarter pops ~500ns worth,
                    # so PE stays packed without ever head-blocking the DVE
                    # scans behind a multi-us conv excursion.
                    filler = collections.deque()
                    pump_allow = [0.0]

                    def pump(add):
                        # allowance ledger: each quarter banks ~the spine's
                        # natural PE slack (scan 1.19us - fill 0.88us); a
                        # unit is popped only once fully funded, so filler
                        # excursions never starve the scan cadence
                        pump_allow[0] += add
                        while filler and pump_allow[0] >= filler[0][0]:
                            ns, fn = filler.popleft()
                            fn()
                            pump_allow[0] -= ns

                    def push_gwork():
                        for c8 in range(NCH):
                            for m in range(2):
                                filler.append(
                                    (230.0,
                                     lambda c8=c8, m=m: g_chunk(c8, m)))
                        for m in range(2):
                            for grp in range(4):
                                filler.append(
                                    (450.0,
                                     lambda m=m, grp=grp:
                                     g_transpose(m, grp)))

                    pend_gather = []
                    s_dram = [d4.tile([JW], F32, tag="sdq", name="sdq"),
                              d4.tile([JW], F32, tag="sdk", name="sdk")]

                    def spine_mjt(m, jt, use_dve_count, tidx):
                        # One match x one 128-j tile: 8 i-chunk matmul pairs
                        # fill [128,1024] PSUM quarters; each quarter is
                        # consumed by a chained DVE prefix-max scan into an
                        # SBUF scratch row P (initial = previous quarter's
                        # last column), then idx = #(P_i < S) via one ACT
                        # Sign-accum pass (or a DVE 2x is_lt pass on tiles
                        # where ACT is the busier engine).  The f32->i32
                        # cast runs on DVE where copies are ~free.
                        scr = ssb.tile([128, HW], F32, tag="scr",
                                       name="scr", bufs=NSCR)
                        ro = m * C
                        js = slice(jt * 128, (jt + 1) * 128)
                        ax_t = axq_t if m == 0 else axk_t
                        for q in range(4):
                            ps = sps.tile([128, 1024], F32,
                                          tag=f"sp{(4 * tidx + q) % 3}",
                                          name="sp")
                            for h in range(2):
                                ch = q * 2 + h
                                cs = slice(ch * 512, (ch + 1) * 512)
                                o = slice(h * 512, (h + 1) * 512)
                                nc.tensor.matmul(
                                    ps[:, o], vhi_t[ro:ro + C, js],
                                    ahi_t[ro:ro + C, cs],
                                    start=True, stop=False,
                                    tile_position=(ro, 0))
                                nc.tensor.matmul(
                                    ps[:, o], vx_t[:, js],
                                    ax_t[:, cs],
                                    start=False, stop=True)
                            qs = slice(q * 1024, (q + 1) * 1024)
                            init = NEG if q == 0 else \
                                scr[:, q * 1024 - 1:q * 1024]
                            nc.vector.tensor_tensor_scan(
                                out=scr[:, qs], data0=ps[:],
                                data1=scan_dummy[:], initial=init,
                                op0=OP.max, op1=OP.bypass)
                            # bank this quarter's PE slack into the filler
                            # allowance
                            pump(340.0)
                        S = s_all[m][:, jt:jt + 1]
                        nc.gpsimd.tensor_copy(S, scr[:, HW - 1:HW])
                        # stream S out per tile (partition-major 4B
                        # descriptors); the group's s_bc broadcast then has
                        # no transpose / staging chain to wait on
                        nc.sync.dma_start(
                            out=s_dram[m][jt * 128:(jt + 1) * 128],
                            in_=S)
                        nia = ssm.tile([128, 1], F32, tag="nia",
                                       name="nia")
                        if use_dve_count:
                            # 2x-mode count on DVE (all-SBUF fp32)
                            nc.vector.tensor_scalar(
                                out=scr[:], in0=scr[:],
                                scalar1=scr[:, HW - 1:HW], scalar2=0.0,
                                op0=OP.is_lt, op1=OP.add,
                                accum_out=nia[:])
                        else:
                            # count on ACT: idx = sum(sign(S - P))
                            nc.scalar.activation(
                                scr[:], scr[:], AF.Sign,
                                bias=scr[:, HW - 1:HW], scale=-1.0,
                                accum_out=nia[:])
                        nc.vector.tensor_copy(
                            idx_all[m][:, jt:jt + 1], nia[:])

                        def fire(m=m, jt=jt):
                            nc.gpsimd.indirect_dma_start(
                                out=gtile[m][:, jt * C:(jt + 1) * C],
                                out_offset=None,
                                in_=gt_dram[m][:],
                                in_offset=bass.IndirectOffsetOnAxis(
                                    ap=idx_all[m][:, jt:jt + 1], axis=0),
                                bounds_check=HW - 1, oob_is_err=False)
                        pend_gather.append(fire)

                    groups = [list(range(4 * g, min(4 * g + 4, JT)))
                              for g in range(5)]

                    def s_group_m(g, m):
                        jts = groups[g]
                        n0 = jts[0] * 128
                        n1 = (jts[-1] + 1) * 128
                        nc.sync.dma_start(
                            out=s_bc[m * C:(m + 1) * C, n0:n1],
                            in_=s_dram[m][None, n0:n1]
                            .to_broadcast((C, n1 - n0)))

                    c1state = {}

                    def conv1_u1(cn, m):
                        # conv1 runs per-match sequentially through one PSUM
                        # bank: m's half is fully evacuated (u3's multiply)
                        # before the other match's start=True re-zeroes it
                        n0 = cn * 512
                        n1 = min(n0 + 512, JW)
                        psm = cvp.tile([128, 512], F32, tag="cva",
                                       name="cva")
                        c1state[cn] = psm
                        nc.tensor.matmul(
                            psm[m * C:(m + 1) * C, 0:n1 - n0],
                            w1vt_t[:], vhi_t[0:C, n0:n1],
                            start=True, stop=False,
                            tile_position=(0, m * C))

                    def conv1_u2(cn, m, lo=0, hi=4):
                        jts = list(range(4 * cn, min(4 * cn + 4, JT)))
                        psm = c1state[cn]
                        for i, jt in list(enumerate(jts))[lo:hi]:
                            if m == 0:
                                nc.tensor.matmul(
                                    psm[0:C, i * 128:(i + 1) * 128],
                                    gtile[m][:, jt * C:(jt + 1) * C],
                                    ident[:], is_transpose=True,
                                    start=False, stop=(jt == jts[-1]))
                            else:
                                nc.tensor.matmul(
                                    psm[C:128,
                                        i * 128:(i + 1) * 128],
                                    gtile[m][:, jt * C:(jt + 1) * C],
                                    ident[:],
                                    start=False, stop=(jt == jts[-1]),
                                    tile_position=(0, C))

                    def conv1_u3(cn, m):
                        n0 = cn * 512
                        n1 = min(n0 + 512, JW)
                        psm = c1state.pop(cn)
                        x0 = n0 // H
                        nx = (n1 - n0) // H
                        nc.vector.tensor_tensor(
                            out=fa3[m * C:(m + 1) * C,
                                    x0:x0 + nx, 1:H + 1],
                            in0=psm[m * C:(m + 1) * C, 0:n1 - n0],
                            in1=s_bc[m * C:(m + 1) * C, n0:n1],
                            op=OP.mult)

                    c2state = {}

                    def conv2_tap(q, t):
                        # one 3x3 tap of one 8-row output quarter (needs
                        # conv1 chunks q and q+1 for its fused-row window)
                        half = q % 2
                        co = slice(half * C, (half + 1) * C)
                        if t == 0:
                            c2state[q] = cvp.tile(
                                [128, 512], F32, tag="c2", name="c2q")
                        psm = c2state[q]
                        ox = 1 + q * 8
                        dx, dy = t // 3, t % 3
                        ra = fa3[:, ox + dx - 1:ox + dx + 7, dy:dy + H]
                        rb = fb3[:, ox + dx - 1:ox + dx + 7, dy:dy + H]
                        nc.tensor.matmul(
                            psm[co, :],
                            w2a_t[:, t * C:(t + 1) * C], ra,
                            start=(t == 0), stop=False,
                            tile_position=(0, half * C))
                        nc.tensor.matmul(
                            psm[co, :],
                            w2b_t[:, t * C:(t + 1) * C], rb,
                            start=False, stop=(t == 8),
                            tile_position=(0, half * C))

                    def conv2_fin(q):
                        half = q % 2
                        co = slice(half * C, (half + 1) * C)
                        psm = c2state.pop(q)
                        ob = slice((q // 2) * 512, (q // 2) * 512 + 512)
                        nc.scalar.activation(
                            out_sb[co, ob],
                            psm[co, :], AF.Relu,
                            bias=bnb_t[co, 0:1], scale=bna_t[co, 0:1])
                        y3 = yout[:].rearrange("c (x y) -> c x y", y=H)
                        nc.sync.dma_start(
                            out=y3[:, q * 8:q * 8 + 8, :],
                            in_=out_sb[co, ob]
                            .rearrange("c (x y) -> c x y", y=H))

                    def push_s_group(g):
                        for m in range(2):
                            filler.append(
                                (30.0, lambda m=m: s_group_m(g, m)))

                    def push_conv1(cn):
                        nj = len(groups[cn])
                        for m in range(2):
                            filler.append(
                                (215.0, lambda m=m: conv1_u1(cn, m)))
                            filler.append(
                                ((110.0 if m == 0 else 215.0) * nj,
                                 lambda m=m: conv1_u2(cn, m)))
                            filler.append(
                                (40.0, lambda m=m: conv1_u3(cn, m)))

                    def push_conv2(q, taps=range(9), fin=True):
                        for t in taps:
                            filler.append(
                                (430.0, lambda t=t: conv2_tap(q, t)))
                        if fin:
                            filler.append((60.0, lambda: conv2_fin(q)))

                    def flush_gathers(keep=0):
                        while len(pend_gather) > keep:
                            pend_gather.pop(0)()

                    push_gwork()
                    tidx = 0
                    for g in range(5):
                        for jt in groups[g]:
                            for m in range(2):
                                # the last j-tile counts on DVE so its idx
                                # lands right after its scans (the ACT queue
                                # would add ~2us to the drain tail)
                                spine_mjt(m, jt, jt == JT - 1, tidx)
                                tidx += 1
                                # gt_dram is complete after group 0; hold
                                # gathers until then so Pool never stalls
                                flush_gathers(keep=2 if g >= 1 else 99)
                                if g == 4:
                                    pump(900.0)

                        if g >= 1:
                            push_s_group(g - 1)
                            push_conv1(g - 1)
                        if g >= 2 and g - 2 != 2:
                            push_conv2(g - 2)
                    flush_gathers(keep=0)
                    pump(1e12)
                    # hand-rolled drain: conv2 quarter 2 (held back) fills
                    # PE between the last chunk's latency-chain steps so the
                    # tail never drops out of the fast PE p-state
                    s_group_m(4, 0)
                    s_group_m(4, 1)
                    conv1_u1(4, 0)
                    for t in range(5):
                        conv2_tap(2, t)
                    conv1_u2(4, 0)
                    conv1_u3(4, 0)
                    conv1_u1(4, 1)
                    for t in range(5, 9):
                        conv2_tap(2, t)
                    conv2_fin(2)
                    conv1_u2(4, 1)
                    conv1_u3(4, 1)
                    for t in range(9):
                        conv2_tap(3, t)
                    conv2_fin(3)
                    pump(1e12)

    nc.finalize()
    return nc


_NC_CACHE = None


def _get_nc():
    global _NC_CACHE
    if _NC_CACHE is None:
        _NC_CACHE = _build_nc()
    return _NC_CACHE


def _bf16_split(x):
    hi = x.astype(ml_dtypes.bfloat16)
    lo = (x - hi.astype(np.float32)).astype(ml_dtypes.bfloat16)
    return hi, lo


def _host_prep(inputs):
    V = np.ascontiguousarray(inputs["V_rgb"], dtype=np.float32)
    K = np.ascontiguousarray(inputs["K_dep"], dtype=np.float32)
    Q = np.ascontiguousarray(inputs["Q_flo"], dtype=np.float32)
    w1 = np.asarray(inputs["conv1_w"], dtype=np.float32)[:, :, 0, 0]
    b1 = np.asarray(inputs["conv1_b"], dtype=np.float32)
    w2 = np.asarray(inputs["conv2_w"], dtype=np.float32)
    b2 = np.asarray(inputs["conv2_b"], dtype=np.float32)
    g = np.asarray(inputs["bn_gamma"], dtype=np.float32)
    be = np.asarray(inputs["bn_beta"], dtype=np.float32)
    mu = np.asarray(inputs["bn_mean"], dtype=np.float32)
    var = np.asarray(inputs["bn_var"], dtype=np.float32)

    w1vt = np.ascontiguousarray(w1[:, :C].T).astype(ml_dtypes.bfloat16)
    w1tt1 = np.ascontiguousarray(w1[:, C:].T)
    w1tt = np.concatenate([w1tt1, w1tt1], axis=0).astype(ml_dtypes.bfloat16)
    w2a = np.zeros((128, 9 * C), np.float32)
    w2b = np.zeros((C, 9 * C), np.float32)
    for t in range(9):
        dx, dy = t // 3, t % 3
        lhsT = w2[:, :, dx, dy].T                     # [192, 64]
        w2a[:, t * C:(t + 1) * C] = lhsT[0:128]
        w2b[:, t * C:(t + 1) * C] = lhsT[128:192]
    w2a = w2a.astype(ml_dtypes.bfloat16)
    w2b = w2b.astype(ml_dtypes.bfloat16)
    bna = g / np.sqrt(var + BN_EPS)
    bnb = be + (b2 - mu) * bna
    bna2 = np.ascontiguousarray(np.concatenate([bna, bna])[:, None])
    bnb2 = np.ascontiguousarray(np.concatenate([bnb, bnb])[:, None])

    in_maps = []
    for core in range(N_CORES):
        b, half = core // 2, core % 2
        x0 = half * (W // 2)
        vw = np.zeros((C, WROWS, H), np.float32)
        lo = x0 - 1
        hi = x0 + W // 2 + 1
        slo, shi = max(lo, 0), min(hi, W)
        vw[:, slo - lo:slo - lo + (shi - slo), :] = V[b, :, slo:shi, :]
        vw = vw.reshape(C, JW)
        aq = Q[b].reshape(C, HW)
        ak = K[b].reshape(C, HW)
        aq1, aq2 = _bf16_split(aq)
        ak1, ak2 = _bf16_split(ak)
        v1, v2 = _bf16_split(vw)
        ahi = np.concatenate([aq1, ak1], axis=0)      # [128, HW]
        axq = np.concatenate([aq2, aq1], axis=0)
        axk = np.concatenate([ak2, ak1], axis=0)
        vhi = np.concatenate([v1, v1], axis=0)        # [128, JW]
        vx = np.concatenate([v1, v2], axis=0)
        in_maps.append({
            "ahi": np.ascontiguousarray(ahi),
            "axq": np.ascontiguousarray(axq),
            "axk": np.ascontiguousarray(axk),
            "vhi": np.ascontiguousarray(vhi),
            "vx": np.ascontiguousarray(vx),
            "vwin": np.ascontiguousarray(vw.astype(ml_dtypes.bfloat16)),
            "w1vt": w1vt,
            "w1tt": w1tt,
            "b1d": np.ascontiguousarray(b1[:, None]),
            "w2ad": w2a,
            "w2bd": w2b,
            "bnad": bna2,
            "bnbd": bnb2,
        })
    return in_maps


def kernel(**inputs):
    nc = _get_nc()
    in_maps = _host_prep(inputs)
    res = bass_utils.run_bass_kernel_spmd(
        nc, in_maps, core_ids=list(range(N_CORES)))
    y = np.zeros((B, C, W, H), np.float32)
    for core in range(N_CORES):
        b, half = core // 2, core % 2
        x0 = half * (W // 2)
        y[b, :, x0:x0 + W // 2, :] = \
            res.results[core]["y"].reshape(C, OUT_ROWS, H)
    return y



# revision 74
# speedup vs baseline: 1.0021x; 1.0021x over previous
"""Trainium2 Bass kernel for nn_Attention_block (retrieval_knn).

Reference (per sample b, match A in {Q_flo, K_dep}, V = V_rgb):
  T[i,j] = <A[:,i], V[:,j]>          [4096, 4096] score matrix
  S[j] = max_i T ; idx[j] = argmax_i T
  C = conv1x1([V; A[:, idx]]) * S    (conv1: 128->64)
  fused = [C_v, C_k, V]              (192 ch)
  y = relu(BN(conv3x3(fused)))       (conv2: 192->64, pad 1)

Sharding: 8 cores = 4 samples x 2 W-halves (pure data parallel; each core
takes a 1-row halo each side of its half for the 3x3 conv and computes its
2176 j-columns against the full 4096-long i axis).

Device-side structure per core (PSUM-resident argmax spine; no ACT
evacuation pass):
  - G-trick: gather commutes with conv1's TA half:
      conv1([V;TA]) + b1 = W1v@V + (W1t@A + b1)[:, idx]
    G' = W1t@A + b1 is computed once per match, transposed into DRAM
    [4096, 64], and argmax rows are fetched by indirect-DMA gather.
  - Scores must be fp32-exact: smallest top-2 score gap here is ~1.5e-4 and
    the reference argmax is f32.  Plain fp32 matmuls cost 4 PE cycles/row
    and fp32r is TF32 (10-bit - flips argmaxes), so the spine uses an exact
    bf16 split computed on the host: A = A1 + A2, V = V1 + V2 (hi/lo bf16
    pairs); T = V1'A1 + (V1'A2 + V2'A1), where the two cross terms fold
    into ONE K=128 matmul by stacking [V1;V2] against [A2;A1] (PE cost is
    per output row, independent of K) - 2 bf16 matmuls/chunk instead of 1
    fp32 at 4 cyc/row.  |T~ - T| ~ 3e-5, 5x below the min gap.
  - Argmax spine, m-major per 128-j tile: T never leaves PSUM (killing the
    141us ACT evacuation pass of the previous design).  Each [128,1024]
    PSUM quarter (2 chunk-matmul pairs) is consumed by a chained DVE
    prefix-max scan (tensor_tensor_scan max/bypass, initial = previous
    quarter's last column, an AP) writing an SBUF scratch row P [128,4096];
    data1 points at a pre-zeroed dummy since src0/src1 cannot both be PSUM.
    S = P[:,-1] (Pool copies it into s_all, and a per-tile partition-major
    DMA streams it to s_dram for the group's s_bc broadcast), and
    idx = #(P_i < S): one ACT Sign pass (scale=-1, bias=S) with the sum
    accumulator - exact, first-occurrence ties.  The last j-tile counts on
    DVE instead (2x-mode is_lt tensor_scalar, 2.2us vs 3.8us) so the drain
    tail isn't gated on the ACT queue.  (Pool/gpsimd cannot run any
    TensorScalarPtr-class op - scans included - on the real compile path:
    walrus rejects them, so Pool only gets gathers/copies/memsets.)
    Engine busy: DVE ~172us (bottleneck), PE ~147us, ACT ~140us, Pool ~45.
  - Fine-grained software pipelining: all non-spine PE work (G' chunks and
    transposes, conv1 per-match passes, conv2 single taps, s_group
    broadcasts) is chopped into ~0.2-0.9us closures on a deque; every spine
    quarter banks 340ns of allowance and pops fully-funded units, so PE
    stays packed inside the scans' per-quarter slack and never head-blocks
    the DVE scan cadence behind a multi-us conv excursion.  Gathers are
    deferred 2 match-tiles; the drain tail is hand-rolled with conv2
    quarter 2 interleaved into the last chunk's latency chain to hold the
    fast PE p-state.  ORDERING IS CORRECTNESS-CRITICAL in two places the
    Tile framework does not track: gt_dram (G' transposes' DMAs must land
    before the first gathers fire at the g==1 flush) and fa3 (conv1 chunk
    q+1's units must be emitted before conv2 quarter q's taps).
  - conv1 / conv2 / G' matmuls run in bf16 (value paths; ~2.9e-3 rel err,
    well under the 2e-2 gate).  The G'->transpose->gather->conv1-TA chain
    stays fp32 (transposes must dtype-match their data, and a 16-bit
    transpose output cannot accumulate into an fp32 PSUM group).
  - PSUM (8 banks): spine 3x[128,1024] quarters rotating (6) + cva (conv1 +
    G' chunks, per-match sequential) + c2 (conv2 + G' transpose staging).
    Sharing one bank between two concurrent 64-partition accumulation
    groups passes the simulator but breaks on hardware (start=True zeroes
    the whole region) - sequential per-match reuse with full evacuation
    between matches is safe and is what cva/c2 do.
"""

import collections

import numpy as np
import ml_dtypes

import concourse.bass as bass
import concourse.bacc as bacc
import concourse.mybir as mybir
from concourse.tile import TileContext
from concourse import bass_utils
from concourse.masks import make_identity

F32 = mybir.dt.float32
BF16 = mybir.dt.bfloat16
I32 = mybir.dt.int32
AF = mybir.ActivationFunctionType
OP = mybir.AluOpType

B, C, W, H = 4, 64, 64, 64
HW = W * H                     # 4096
BN_EPS = 1e-5
N_CORES = 8
WROWS = W // 2 + 2             # 34 window rows (half + 1-row halo each side)
JW = WROWS * H                 # 2176 j-columns per core
JT = JW // 128                 # 17 j-tiles
NCH = HW // 512                # 8 i-chunks
OUT_ROWS = W // 2              # 32 interior rows per core
YPAD = H + 2                   # 66 padded y positions in fused layout

NEG = -3.0e38


def _build_nc():
    nc = bacc.Bacc("TRN2", target_bir_lowering=False)

    # Exact bf16 split pairs (X = X1 + X2 with X2 itself bf16-exact to
    # ~2^-17 rel).  T = V1'A1 + (V1'A2 + V2'A1): the two cross terms are
    # computed as ONE K=128 matmul by stacking [V1;V2] against [A2;A1] -
    # PE cost is per output row, independent of K.
    #   ahi: rows 0:64 = A1_q, rows 64:128 = A1_k
    #   axq/axk: rows 0:64 = A2_m, rows 64:128 = A1_m  (match m)
    #   vhi: V1 duplicated on both row halves; vx: rows 0:64 V1, 64:128 V2
    ahi = nc.dram_tensor("ahi", [128, HW], BF16, kind="ExternalInput")
    axq = nc.dram_tensor("axq", [128, HW], BF16, kind="ExternalInput")
    axk = nc.dram_tensor("axk", [128, HW], BF16, kind="ExternalInput")
    vhi = nc.dram_tensor("vhi", [128, JW], BF16, kind="ExternalInput")
    vx = nc.dram_tensor("vx", [128, JW], BF16, kind="ExternalInput")
    vwin = nc.dram_tensor("vwin", [C, JW], BF16, kind="ExternalInput")
    w1vt = nc.dram_tensor("w1vt", [C, C], BF16, kind="ExternalInput")
    w1tt = nc.dram_tensor("w1tt", [128, C], BF16, kind="ExternalInput")
    b1d = nc.dram_tensor("b1d", [C, 1], F32, kind="ExternalInput")
    w2ad = nc.dram_tensor("w2ad", [128, 9 * C], BF16, kind="ExternalInput")
    w2bd = nc.dram_tensor("w2bd", [C, 9 * C], BF16, kind="ExternalInput")
    bnad = nc.dram_tensor("bnad", [128, 1], F32, kind="ExternalInput")
    bnbd = nc.dram_tensor("bnbd", [128, 1], F32, kind="ExternalInput")
    yout = nc.dram_tensor("y", [C, OUT_ROWS * H], F32, kind="ExternalOutput")

    if True:
      with TileContext(nc) as tc:
        with tc.tile_pool(name="persist", bufs=1) as pp:
            ahi_t = pp.tile([128, HW], BF16)
            axq_t = pp.tile([128, HW], BF16)
            axk_t = pp.tile([128, HW], BF16)
            vhi_t = pp.tile([128, JW], BF16)
            vx_t = pp.tile([128, JW], BF16)
            w1vt_t = pp.tile([C, C], BF16)
            w1tt_t = pp.tile([128, C], BF16)
            b1_t = pp.tile([C, 1], F32)
            w2a_t = pp.tile([128, 9 * C], BF16)
            w2b_t = pp.tile([C, 9 * C], BF16)
            bna_t = pp.tile([128, 1], F32)
            bnb_t = pp.tile([128, 1], F32)
            ident = pp.tile([128, 128], F32)
            gtile = [pp.tile([128, JT * C], F32, tag="gtq", name="gtq_t"),
                     pp.tile([128, JT * C], F32, tag="gtk", name="gtk_t")]
            s_all = [pp.tile([128, JT], F32, tag="sq", name="sq_t"),
                     pp.tile([128, JT], F32, tag="sk", name="sk_t")]
            idx_all = [pp.tile([128, JT], I32, tag="idxq", name="idxq_t"),
                       pp.tile([128, JT], I32, tag="idxk", name="idxk_t")]
            s_bc = pp.tile([128, JW], F32)     # rows 0:64 Sq, 64:128 Sk
            fused_a = pp.tile([128, WROWS * YPAD], BF16)  # C_v / C_k
            fused_b = pp.tile([C, WROWS * YPAD], BF16)    # V, y-padded
            out_sb = pp.tile([128, OUT_ROWS * H // 2], F32)

            # SP.SEQ issues one DMA per ~650ns, so the head of the queue is
            # precious: the first spine quarter's operands go first (vhi/vx
            # slices ride the ACT queue in parallel), then everything else
            # as few big transfers.
            nc.sync.dma_start(out=w1tt_t[:], in_=w1tt[:])
            nc.sync.dma_start(out=b1_t[:], in_=b1d[:])
            nc.sync.dma_start(out=ahi_t[:, 0:1024], in_=ahi[:, 0:1024])
            nc.sync.dma_start(out=axq_t[:, 0:1024], in_=axq[:, 0:1024])
            nc.sync.dma_start(out=vhi_t[:, 0:256], in_=vhi[:, 0:256])
            nc.sync.dma_start(out=vx_t[:, 0:256], in_=vx[:, 0:256])
            nc.sync.dma_start(out=axk_t[:, 0:1024], in_=axk[:, 0:1024])
            for cq in range(1, 4):
                cs = slice(cq * 1024, (cq + 1) * 1024)
                nc.sync.dma_start(out=ahi_t[:, cs], in_=ahi[:, cs])
                nc.sync.dma_start(out=axq_t[:, cs], in_=axq[:, cs])
                nc.sync.dma_start(out=axk_t[:, cs], in_=axk[:, cs])
            nc.sync.dma_start(out=vhi_t[:, 256:JW], in_=vhi[:, 256:JW])
            nc.sync.dma_start(out=vx_t[:, 256:JW], in_=vx[:, 256:JW])
            nc.sync.dma_start(out=w1vt_t[:], in_=w1vt[:])
            nc.sync.dma_start(out=w2a_t[:], in_=w2ad[:])
            nc.sync.dma_start(out=w2b_t[:], in_=w2bd[:])
            nc.sync.dma_start(out=bna_t[:], in_=bnad[:])
            nc.sync.dma_start(out=bnb_t[:], in_=bnbd[:])
            make_identity(nc, ident[:])

            # the first DVE scan reads scan_dummy (bypassed data1): its
            # memset must head the Pool queue, ahead of the big gtile/fused
            # memsets, or the spine start stalls ~5us behind them
            scan_dummy = pp.tile([128, 1024], F32)
            nc.gpsimd.memset(scan_dummy[:], 0.0)

            fb3 = fused_b[:].rearrange("c (x y) -> c x y", y=YPAD)
            nc.gpsimd.memset(fused_b[:], 0.0)
            nc.sync.dma_start(
                out=fb3[:, :, 1:H + 1],
                in_=vwin[:].rearrange("c (x y) -> c x y", y=H))
            nc.gpsimd.memset(gtile[0][:], 0.0)
            nc.gpsimd.memset(gtile[1][:], 0.0)

            with tc.tile_pool(name="gdram", bufs=1, space="DRAM") as gdr:
                gt_dram = [gdr.tile([HW, C], F32, tag="gtdq", name="gtdq_t"),
                           gdr.tile([HW, C], F32, tag="gtdk", name="gtdk_t")]

                # ---- Phases 3-6 interleaved ----
                fa3 = fused_a[:].rearrange("c (x y) -> c x y", y=YPAD)
                nc.gpsimd.memset(fa3[:, :, 0:1], 0.0)
                nc.gpsimd.memset(fa3[:, :, YPAD - 1:YPAD], 0.0)
                with tc.tile_pool(name="sp_ps", bufs=1, space="PSUM") as sps, \
                     tc.tile_pool(name="sp_sb", bufs=2) as ssb, \
                     tc.tile_pool(name="sp_sm", bufs=4) as ssm, \
                     tc.tile_pool(name="cv_ps", bufs=1, space="PSUM") as cvp, \
                     tc.tile_pool(name="s4_sb", bufs=2) as s4, \
                     tc.tile_pool(name="s4_dram", bufs=1, space="DRAM") as d4:

                    NSCR = 3

                    # ---- Phase 1+2: G' = W1t @ A1 + b1; transpose to DRAM.
                    # Emitted as closures interleaved into spine group 0 so
                    # the startup isn't serial (spine doesn't depend on G').
                    g_sbs = [pp.tile([C, HW], F32, tag="gsbq", name="gsbq"),
                             pp.tile([C, HW], F32, tag="gsbk", name="gsbk")]

                    def g_chunk(c8, m):
                        # sequential per-match through one PSUM bank (the
                        # two 64-partition halves cannot share a bank
                        # concurrently: start=True zeroes the whole region)
                        pm = cvp.tile([C, 512], F32, tag="cva", name="gm")
                        ro = m * C
                        nc.tensor.matmul(
                            pm[:], w1tt_t[ro:ro + C, :],
                            ahi_t[ro:ro + C, c8 * 512:(c8 + 1) * 512],
                            start=True, stop=True,
                            tile_position=(ro, 0))
                        nc.scalar.activation(
                            g_sbs[m][:, c8 * 512:(c8 + 1) * 512],
                            pm[:],
                            AF.Identity, bias=b1_t[:, 0:1], scale=1.0)

                    def g_transpose(m, grp):
                        g_sb = g_sbs[m]
                        pst = cvp.tile([128, 512], F32, tag="c2",
                                       name="gtr")
                        stg = pp.tile([128, 512], F32, tag="stg")
                        for t in range(8):
                            blk = grp * 8 + t
                            nc.tensor.matmul(
                                pst[:, t * C:(t + 1) * C],
                                g_sb[:, blk * 128:(blk + 1) * 128],
                                ident[0:C, 0:C], is_transpose=True,
                                start=True, stop=True)
                        nc.scalar.copy(stg[:], pst[:])
                        nc.sync.dma_start(
                            out=gt_dram[m][:]
                            .rearrange("(g p) c -> p g c", p=128)
                            [:, grp * 8:(grp + 1) * 8, :],
                            in_=stg[:].rearrange("p (g c) -> p g c", c=C))

                    # Fine-grained software pipelining: every non-spine PE
                    # task (G', conv1, s_group, conv2) is chopped into ~0.5us
                    # units on a deque; each spine quarter pops ~500ns worth,
                    # so PE stays packed without ever head-blocking the DVE
                    # scans behind a multi-us conv excursion.
                    filler = collections.deque()
                    pump_allow = [0.0]

                    def pump(add):
                        # allowance ledger: each quarter banks ~the spine's
                        # natural PE slack (scan 1.19us - fill 0.88us); a
                        # unit is popped only once fully funded, so filler
                        # excursions never starve the scan cadence
                        pump_allow[0] += add
                        while filler and pump_allow[0] >= filler[0][0]:
                            ns, fn = filler.popleft()
                            fn()
                            pump_allow[0] -= ns

                    def push_gwork():
                        for c8 in range(NCH):
                            for m in range(2):
                                filler.append(
                                    (230.0,
                                     lambda c8=c8, m=m: g_chunk(c8, m)))
                        for m in range(2):
                            for grp in range(4):
                                filler.append(
                                    (450.0,
                                     lambda m=m, grp=grp:
                                     g_transpose(m, grp)))

                    pend_gather = []
                    s_dram = [d4.tile([JW], F32, tag="sdq", name="sdq"),
                              d4.tile([JW], F32, tag="sdk", name="sdk")]

                    def spine_mjt(m, jt, use_dve_count, tidx):
                        # One match x one 128-j tile: 8 i-chunk matmul pairs
                        # fill [128,1024] PSUM quarters; each quarter is
                        # consumed by a chained DVE prefix-max scan into an
                        # SBUF scratch row P (initial = previous quarter's
                        # last column), then idx = #(P_i < S) via one ACT
                        # Sign-accum pass (or a DVE 2x is_lt pass on tiles
                        # where ACT is the busier engine).  The f32->i32
                        # cast runs on DVE where copies are ~free.
                        scr = ssb.tile([128, HW], F32, tag="scr",
                                       name="scr", bufs=NSCR)
                        ro = m * C
                        js = slice(jt * 128, (jt + 1) * 128)
                        ax_t = axq_t if m == 0 else axk_t
                        for q in range(4):
                            ps = sps.tile([128, 1024], F32,
                                          tag=f"sp{(4 * tidx + q) % 3}",
                                          name="sp")
                            for h in range(2):
                                ch = q * 2 + h
                                cs = slice(ch * 512, (ch + 1) * 512)
                                o = slice(h * 512, (h + 1) * 512)
                                nc.tensor.matmul(
                                    ps[:, o], vhi_t[ro:ro + C, js],
                                    ahi_t[ro:ro + C, cs],
                                    start=True, stop=False,
                                    tile_position=(ro, 0))
                                nc.tensor.matmul(
                                    ps[:, o], vx_t[:, js],
                                    ax_t[:, cs],
                                    start=False, stop=True)
                            qs = slice(q * 1024, (q + 1) * 1024)
                            init = NEG if q == 0 else \
                                scr[:, q * 1024 - 1:q * 1024]
                            nc.vector.tensor_tensor_scan(
                                out=scr[:, qs], data0=ps[:],
                                data1=scan_dummy[:], initial=init,
                                op0=OP.max, op1=OP.bypass)
                            # bank this quarter's PE slack into the filler
                            # allowance
                            pump(355.0)
                        S = s_all[m][:, jt:jt + 1]
                        nc.gpsimd.tensor_copy(S, scr[:, HW - 1:HW])
                        # stream S out per tile (partition-major 4B
                        # descriptors); the group's s_bc broadcast then has
                        # no transpose / staging chain to wait on
                        nc.sync.dma_start(
                            out=s_dram[m][jt * 128:(jt + 1) * 128],
                            in_=S)
                        nia = ssm.tile([128, 1], F32, tag="nia",
                                       name="nia")
                        if use_dve_count:
                            # 2x-mode count on DVE (all-SBUF fp32)
                            nc.vector.tensor_scalar(
                                out=scr[:], in0=scr[:],
                                scalar1=scr[:, HW - 1:HW], scalar2=0.0,
                                op0=OP.is_lt, op1=OP.add,
                                accum_out=nia[:])
                        else:
                            # count on ACT: idx = sum(sign(S - P))
                            nc.scalar.activation(
                                scr[:], scr[:], AF.Sign,
                                bias=scr[:, HW - 1:HW], scale=-1.0,
                                accum_out=nia[:])
                        nc.vector.tensor_copy(
                            idx_all[m][:, jt:jt + 1], nia[:])

                        def fire(m=m, jt=jt):
                            nc.gpsimd.indirect_dma_start(
                                out=gtile[m][:, jt * C:(jt + 1) * C],
                                out_offset=None,
                                in_=gt_dram[m][:],
                                in_offset=bass.IndirectOffsetOnAxis(
                                    ap=idx_all[m][:, jt:jt + 1], axis=0),
                                bounds_check=HW - 1, oob_is_err=False)
                        pend_gather.append(fire)

                    groups = [list(range(4 * g, min(4 * g + 4, JT)))
                              for g in range(5)]

                    def s_group_m(g, m):
                        jts = groups[g]
                        n0 = jts[0] * 128
                        n1 = (jts[-1] + 1) * 128
                        nc.sync.dma_start(
                            out=s_bc[m * C:(m + 1) * C, n0:n1],
                            in_=s_dram[m][None, n0:n1]
                            .to_broadcast((C, n1 - n0)))

                    c1state = {}

                    def conv1_u1(cn, m):
                        # conv1 runs per-match sequentially through one PSUM
                        # bank: m's half is fully evacuated (u3's multiply)
                        # before the other match's start=True re-zeroes it
                        n0 = cn * 512
                        n1 = min(n0 + 512, JW)
                        psm = cvp.tile([128, 512], F32, tag="cva",
                                       name="cva")
                        c1state[cn] = psm
                        nc.tensor.matmul(
                            psm[m * C:(m + 1) * C, 0:n1 - n0],
                            w1vt_t[:], vhi_t[0:C, n0:n1],
                            start=True, stop=False,
                            tile_position=(0, m * C))

                    def conv1_u2(cn, m, lo=0, hi=4):
                        jts = list(range(4 * cn, min(4 * cn + 4, JT)))
                        psm = c1state[cn]
                        for i, jt in list(enumerate(jts))[lo:hi]:
                            if m == 0:
                                nc.tensor.matmul(
                                    psm[0:C, i * 128:(i + 1) * 128],
                                    gtile[m][:, jt * C:(jt + 1) * C],
                                    ident[:], is_transpose=True,
                                    start=False, stop=(jt == jts[-1]))
                            else:
                                nc.tensor.matmul(
                                    psm[C:128,
                                        i * 128:(i + 1) * 128],
                                    gtile[m][:, jt * C:(jt + 1) * C],
                                    ident[:],
                                    start=False, stop=(jt == jts[-1]),
                                    tile_position=(0, C))

                    def conv1_u3(cn, m):
                        n0 = cn * 512
                        n1 = min(n0 + 512, JW)
                        psm = c1state.pop(cn)
                        x0 = n0 // H
                        nx = (n1 - n0) // H
                        nc.vector.tensor_tensor(
                            out=fa3[m * C:(m + 1) * C,
                                    x0:x0 + nx, 1:H + 1],
                            in0=psm[m * C:(m + 1) * C, 0:n1 - n0],
                            in1=s_bc[m * C:(m + 1) * C, n0:n1],
                            op=OP.mult)

                    c2state = {}

                    def conv2_tap(q, t):
                        # one 3x3 tap of one 8-row output quarter (needs
                        # conv1 chunks q and q+1 for its fused-row window)
                        half = q % 2
                        co = slice(half * C, (half + 1) * C)
                        if t == 0:
                            c2state[q] = cvp.tile(
                                [128, 512], F32, tag="c2", name="c2q")
                        psm = c2state[q]
                        ox = 1 + q * 8
                        dx, dy = t // 3, t % 3
                        ra = fa3[:, ox + dx - 1:ox + dx + 7, dy:dy + H]
                        rb = fb3[:, ox + dx - 1:ox + dx + 7, dy:dy + H]
                        nc.tensor.matmul(
                            psm[co, :],
                            w2a_t[:, t * C:(t + 1) * C], ra,
                            start=(t == 0), stop=False,
                            tile_position=(0, half * C))
                        nc.tensor.matmul(
                            psm[co, :],
                            w2b_t[:, t * C:(t + 1) * C], rb,
                            start=False, stop=(t == 8),
                            tile_position=(0, half * C))

                    def conv2_fin(q):
                        half = q % 2
                        co = slice(half * C, (half + 1) * C)
                        psm = c2state.pop(q)
                        ob = slice((q // 2) * 512, (q // 2) * 512 + 512)
                        nc.scalar.activation(
                            out_sb[co, ob],
                            psm[co, :], AF.Relu,
                            bias=bnb_t[co, 0:1], scale=bna_t[co, 0:1])
                        y3 = yout[:].rearrange("c (x y) -> c x y", y=H)
                        nc.sync.dma_start(
                            out=y3[:, q * 8:q * 8 + 8, :],
                            in_=out_sb[co, ob]
                            .rearrange("c (x y) -> c x y", y=H))

                    def push_s_group(g):
                        for m in range(2):
                            filler.append(
                                (30.0, lambda m=m: s_group_m(g, m)))

                    def push_conv1(cn):
                        nj = len(groups[cn])
                        for m in range(2):
                            filler.append(
                                (215.0, lambda m=m: conv1_u1(cn, m)))
                            filler.append(
                                ((110.0 if m == 0 else 215.0) * nj,
                                 lambda m=m: conv1_u2(cn, m)))
                            filler.append(
                                (40.0, lambda m=m: conv1_u3(cn, m)))

                    def push_conv2(q, taps=range(9), fin=True):
                        for t in taps:
                            filler.append(
                                (430.0, lambda t=t: conv2_tap(q, t)))
                        if fin:
                            filler.append((60.0, lambda: conv2_fin(q)))

                    def flush_gathers(keep=0):
                        while len(pend_gather) > keep:
                            pend_gather.pop(0)()

                    push_gwork()
                    tidx = 0
                    for g in range(5):
                        for jt in groups[g]:
                            for m in range(2):
                                # the last j-tile counts on DVE so its idx
                                # lands right after its scans (the ACT queue
                                # would add ~2us to the drain tail)
                                spine_mjt(m, jt, jt == JT - 1, tidx)
                                tidx += 1
                                # gt_dram is complete after group 0; hold
                                # gathers until then so Pool never stalls
                                flush_gathers(keep=2 if g >= 1 else 99)
                                if g == 4:
                                    pump(900.0)

                        if g >= 1:
                            push_s_group(g - 1)
                            push_conv1(g - 1)
                        if g >= 2 and g - 2 != 2:
                            push_conv2(g - 2)
                    flush_gathers(keep=0)
                    pump(1e12)
                    # hand-rolled drain: conv2 quarter 2 (held back) fills
                    # PE between the last chunk's latency-chain steps so the
                    # tail never drops out of the fast PE p-state
                    s_group_m(4, 0)
                    s_group_m(4, 1)
                    conv1_u1(4, 0)
                    for t in range(5):
                        conv2_tap(2, t)
                    conv1_u2(4, 0)
                    conv1_u3(4, 0)
                    conv1_u1(4, 1)
                    for t in range(5, 9):
                        conv2_tap(2, t)
                    conv2_fin(2)
                    conv1_u2(4, 1)
                    conv1_u3(4, 1)
                    for t in range(9):
                        conv2_tap(3, t)
                    conv2_fin(3)
                    pump(1e12)

    nc.finalize()
    return nc


_NC_CACHE = None


def _get_nc():
    global _NC_CACHE
    if _NC_CACHE is None:
        _NC_CACHE = _build_nc()
    return _NC_CACHE


def _bf16_split(x):
    hi = x.astype(ml_dtypes.bfloat16)
    lo = (x - hi.astype(np.float32)).astype(ml_dtypes.bfloat16)
    return hi, lo


def _host_prep(inputs):
    V = np.ascontiguousarray(inputs["V_rgb"], dtype=np.float32)
    K = np.ascontiguousarray(inputs["K_dep"], dtype=np.float32)
    Q = np.ascontiguousarray(inputs["Q_flo"], dtype=np.float32)
    w1 = np.asarray(inputs["conv1_w"], dtype=np.float32)[:, :, 0, 0]
    b1 = np.asarray(inputs["conv1_b"], dtype=np.float32)
    w2 = np.asarray(inputs["conv2_w"], dtype=np.float32)
    b2 = np.asarray(inputs["conv2_b"], dtype=np.float32)
    g = np.asarray(inputs["bn_gamma"], dtype=np.float32)
    be = np.asarray(inputs["bn_beta"], dtype=np.float32)
    mu = np.asarray(inputs["bn_mean"], dtype=np.float32)
    var = np.asarray(inputs["bn_var"], dtype=np.float32)

    w1vt = np.ascontiguousarray(w1[:, :C].T).astype(ml_dtypes.bfloat16)
    w1tt1 = np.ascontiguousarray(w1[:, C:].T)
    w1tt = np.concatenate([w1tt1, w1tt1], axis=0).astype(ml_dtypes.bfloat16)
    w2a = np.zeros((128, 9 * C), np.float32)
    w2b = np.zeros((C, 9 * C), np.float32)
    for t in range(9):
        dx, dy = t // 3, t % 3
        lhsT = w2[:, :, dx, dy].T                     # [192, 64]
        w2a[:, t * C:(t + 1) * C] = lhsT[0:128]
        w2b[:, t * C:(t + 1) * C] = lhsT[128:192]
    w2a = w2a.astype(ml_dtypes.bfloat16)
    w2b = w2b.astype(ml_dtypes.bfloat16)
    bna = g / np.sqrt(var + BN_EPS)
    bnb = be + (b2 - mu) * bna
    bna2 = np.ascontiguousarray(np.concatenate([bna, bna])[:, None])
    bnb2 = np.ascontiguousarray(np.concatenate([bnb, bnb])[:, None])

    in_maps = []
    for core in range(N_CORES):
        b, half = core // 2, core % 2
        x0 = half * (W // 2)
        vw = np.zeros((C, WROWS, H), np.float32)
        lo = x0 - 1
        hi = x0 + W // 2 + 1
        slo, shi = max(lo, 0), min(hi, W)
        vw[:, slo - lo:slo - lo + (shi - slo), :] = V[b, :, slo:shi, :]
        vw = vw.reshape(C, JW)
        aq = Q[b].reshape(C, HW)
        ak = K[b].reshape(C, HW)
        aq1, aq2 = _bf16_split(aq)
        ak1, ak2 = _bf16_split(ak)
        v1, v2 = _bf16_split(vw)
        ahi = np.concatenate([aq1, ak1], axis=0)      # [128, HW]
        axq = np.concatenate([aq2, aq1], axis=0)
        axk = np.concatenate([ak2, ak1], axis=0)
        vhi = np.concatenate([v1, v1], axis=0)        # [128, JW]
        vx = np.concatenate([v1, v2], axis=0)
        in_maps.append({
            "ahi": np.ascontiguousarray(ahi),
            "axq": np.ascontiguousarray(axq),
            "axk": np.ascontiguousarray(axk),
            "vhi": np.ascontiguousarray(vhi),
            "vx": np.ascontiguousarray(vx),
            "vwin": np.ascontiguousarray(vw.astype(ml_dtypes.bfloat16)),
            "w1vt": w1vt,
            "w1tt": w1tt,
            "b1d": np.ascontiguousarray(b1[:, None]),
            "w2ad": w2a,
            "w2bd": w2b,
            "bnad": bna2,
            "bnbd": bnb2,
        })
    return in_maps


def kernel(**inputs):
    nc = _get_nc()
    in_maps = _host_prep(inputs)
    res = bass_utils.run_bass_kernel_spmd(
        nc, in_maps, core_ids=list(range(N_CORES)))
    y = np.zeros((B, C, W, H), np.float32)
    for core in range(N_CORES):
        b, half = core // 2, core % 2
        x0 = half * (W // 2)
        y[b, :, x0:x0 + W // 2, :] = \
            res.results[core]["y"].reshape(C, OUT_ROWS, H)
    return y

